# revision 3
# baseline (speedup 1.0000x reference)
"""Trainium2 Bass kernel for sum-of-7-box-blurs (k=3..15, edge padding) * base_map.

Math: out = base_map * sum_k 1/(7 k^2) * V_k(H_k(x)) with V_k/H_k k-wide box
sums (edge padding = clamped indexing, handled by host-side padding).

Decomposition (per 114-row output tile, 128 contraction rows):
  acc = M_3 @ h3 + sum_{j in 5..15 step 2} M_j @ (xl_p + xr_p),  p=(j-1)/2
where h3 is the horizontal 3-box sum, xl/xr are column-shifted x (free-dim AP
offsets, free on the PE), and M_j = sum_{k>=j} c_k A_k are 15-wide banded
matrices folded on the host. j=5 pair is summed on DVE, j=7/9 on GPSIMD,
j=11/13/15 fed as two shifted matmuls each -> 10 matmuls per out tile.

Sharding: rows split across 8 cores; halo rows come from host-side padding so
cores are fully independent.
"""

import numpy as np

import concourse.bass as bass
import concourse.mybir as mybir
import concourse.tile as tile
from concourse import bacc, bass_utils

H = W = 4096
NC = 8
RPC = H // NC                 # 512 output rows per core
PAD = 7
PW = W + 2 * PAD              # 4110 padded cols
PR = RPC + 2 * PAD            # 526 padded rows per core
M_TILE = 114                  # out rows per PE tile (114 + 14 = 128 contraction)
ROW_TILES = [(0, 114), (114, 114), (228, 114), (342, 114), (456, 56)]
CHUNK = 2048                  # column chunk for DVE/GPSIMD array ops
K_SIZES = [3, 5, 7, 9, 11, 13, 15]
F32R = mybir.dt.float32r
F32 = mybir.dt.float32


def _weights_np() -> np.ndarray:
    """lhsT matrices [7, 128, M_TILE]: lhsT[j][i, m] = w_j[i - m]."""
    c = {k: 1.0 / (len(K_SIZES) * k * k) for k in K_SIZES}
    wts = np.zeros((7, 128, M_TILE), dtype=np.float64)
    for ji, j in enumerate(K_SIZES):
        w = np.array(
            [sum(c[k] for k in K_SIZES if k >= j and k >= 2 * abs(d - PAD) + 1)
             for d in range(2 * PAD + 1)])
        for m in range(M_TILE):
            wts[ji, m:m + 15, m] = w
    return wts.astype(np.float32)


def _kernel_body(nc, tc, xp_d, bm_d, w_d, out_d):
    add = mybir.AluOpType.add
    sub = mybir.AluOpType.subtract
    mult = mybir.AluOpType.mult

    with (
        tc.tile_pool(name="wpool", bufs=1) as wpool,
        tc.tile_pool(name="xpool", bufs=2) as xpool,
        tc.tile_pool(name="apool", bufs=2) as apool,
        tc.tile_pool(name="bmpool", bufs=2) as bmpool,
        tc.tile_pool(name="opool", bufs=3) as opool,
        tc.tile_pool(name="psum", bufs=3, space="PSUM") as psum_pool,
    ):
        wsb = wpool.tile([128, 7 * M_TILE], F32R)
        for j in range(7):
            nc.sync.dma_start(out=wsb[:, j * M_TILE:(j + 1) * M_TILE], in_=w_d[j])

        for rt, Mt in ROW_TILES:
            Krows = min(128, PR - rt)     # 128, last tile 70
            x_sb = xpool.tile([128, PW], F32R, tag="x")
            nc.sync.dma_start(out=x_sb[:Krows], in_=xp_d[rt:rt + Krows])
            bm_sb = bmpool.tile([128, W], F32, tag="bm")
            nc.sync.dma_start(out=bm_sb[:Mt], in_=bm_d[rt:rt + Mt])
            X = x_sb[:Krows]

            prev_h3 = None
            for ci, co in enumerate(range(0, W, CHUNK)):
                h3 = apool.tile([128, CHUNK], F32R, tag="h3")
                d5 = apool.tile([128, CHUNK], F32R, tag="d5")
                d7 = apool.tile([128, CHUNK], F32R, tag="d7")
                d9 = apool.tile([128, CHUNK], F32R, tag="d9")
                # horizontal 3-box via scan: box(t) = box(t-1) + xp[t+8] - xp[t+5]
                if ci == 0:
                    # f32r tiles are bit-identical f32 for DVE; only the PE
                    # reads them as float32r.
                    with nc.allow_low_precision(reason="f32r == f32 on DVE"):
                        nc.vector.tensor_reduce(
                            out=h3[:Krows, 0:1], in_=X[:, 6:9],
                            axis=mybir.AxisListType.X, op=add)
                    nc.vector.tensor_tensor_scan(
                        out=h3[:Krows, 1:CHUNK],
                        data0=X[:, 9:9 + CHUNK - 1], data1=X[:, 6:6 + CHUNK - 1],
                        initial=h3[:Krows, 0:1], op0=add, op1=sub)
                else:
                    nc.vector.tensor_tensor_scan(
                        out=h3[:Krows, :],
                        data0=X[:, co + 8:co + 8 + CHUNK],
                        data1=X[:, co + 5:co + 5 + CHUNK],
                        initial=prev_h3[:Krows, CHUNK - 1:CHUNK],
                        op0=add, op1=sub)
                prev_h3 = h3
                # delta pairs: d_j = x shifted left + right by p=(j-1)/2
                nc.vector.tensor_tensor(
                    out=d5[:Krows], in0=X[:, co + 5:co + 5 + CHUNK],
                    in1=X[:, co + 9:co + 9 + CHUNK], op=add)
                nc.gpsimd.tensor_tensor(
                    out=d7[:Krows], in0=X[:, co + 4:co + 4 + CHUNK],
                    in1=X[:, co + 10:co + 10 + CHUNK], op=add)
                nc.gpsimd.tensor_tensor(
                    out=d9[:Krows], in0=X[:, co + 3:co + 3 + CHUNK],
                    in1=X[:, co + 11:co + 11 + CHUNK], op=add)

                for half in range(CHUNK // 1024):
                    ps = psum_pool.tile([M_TILE, 1024], F32)
                    for s2 in range(2):
                        lo = half * 1024 + s2 * 512   # chunk-local col
                        gco = co + lo                 # global out col

                        def mm(widx, rhs, start=False, stop=False):
                            nc.tensor.matmul(
                                ps[:Mt, s2 * 512:(s2 + 1) * 512],
                                wsb[:Krows, widx * M_TILE:widx * M_TILE + Mt],
                                rhs, start=start, stop=stop)

                        mm(0, h3[:Krows, lo:lo + 512], start=True)
                        mm(1, d5[:Krows, lo:lo + 512])
                        mm(2, d7[:Krows, lo:lo + 512])
                        mm(3, d9[:Krows, lo:lo + 512])
                        mm(4, X[:, gco + 2:gco + 2 + 512])
                        mm(4, X[:, gco + 12:gco + 12 + 512])
                        mm(5, X[:, gco + 1:gco + 1 + 512])
                        mm(5, X[:, gco + 13:gco + 13 + 512])
                        mm(6, X[:, gco + 0:gco + 0 + 512])
                        mm(6, X[:, gco + 14:gco + 14 + 512], stop=True)

                    osb = opool.tile([M_TILE, 1024], F32, tag="o")
                    oc = co + half * 1024
                    nc.vector.tensor_tensor(
                        out=osb[:Mt], in0=ps[:Mt], in1=bm_sb[:Mt, oc:oc + 1024],
                        op=mult)
                    nc.sync.dma_start(
                        out=out_d[rt:rt + Mt, oc:oc + 1024], in_=osb[:Mt])


def _build():
    nc = bacc.Bacc("TRN2", target_bir_lowering=False, debug=False)
    xp_d = nc.dram_tensor("xp", [PR, PW], F32R, kind="ExternalInput").ap()
    bm_d = nc.dram_tensor("bm", [RPC, W], F32, kind="ExternalInput").ap()
    w_d = nc.dram_tensor("wts", [7, 128, M_TILE], F32R, kind="ExternalInput").ap()
    out_d = nc.dram_tensor("out", [RPC, W], F32, kind="ExternalOutput").ap()
    with tile.TileContext(nc) as tc:
        _kernel_body(nc, tc, xp_d, bm_d, w_d, out_d)
    nc.compile()
    return nc


_CACHE: dict = {}


def _get_nc():
    if "nc" not in _CACHE:
        _CACHE["nc"] = _build()
    return _CACHE["nc"]


def _in_maps(x: np.ndarray, base_map: np.ndarray) -> list[dict]:
    xp = np.pad(x, PAD, mode="edge")
    wts = _weights_np()
    maps = []
    for c in range(NC):
        maps.append({
            "xp": np.ascontiguousarray(xp[c * RPC: c * RPC + PR]),
            "bm": np.ascontiguousarray(base_map[c * RPC:(c + 1) * RPC]),
            "wts": wts,
        })
    return maps


def run(x, base_map, **kwargs) -> tuple[np.ndarray, bass_utils.BassKernelResults]:
    x = np.ascontiguousarray(np.asarray(x), dtype=np.float32)
    base_map = np.ascontiguousarray(np.asarray(base_map), dtype=np.float32)
    nc = _get_nc()
    res = bass_utils.run_bass_kernel_spmd(
        nc, _in_maps(x, base_map), core_ids=list(range(NC)), **kwargs)
    out = np.concatenate([r["out"] for r in res.results], axis=0)
    return out[None, None].astype(np.float32, copy=False), res


def kernel(x, base_map) -> np.ndarray:
    return run(x, base_map)[0]


# revision 4
# speedup vs baseline: 1.0913x; 1.0913x over previous
"""Trainium2 Bass kernel for sum-of-7-box-blurs (k=3..15, edge padding) * base_map.

Math: out = base_map * sum_k 1/(7 k^2) * V_k(H_k(x)) with V_k/H_k k-wide box
sums (edge padding = clamped indexing, handled by host-side padding).

Horizontal delta decomposition (p = (j-1)/2, d_j = x<<p + x>>p column shifts):
  acc = M_3 x + sum_{j in 3..15 step 2} M_j d_j,   M_j = sum_{k>=j} c_k A_k
where M_j are 15-wide banded vertical matrices folded on the host. Per
114-row out tile the vertical mix is a K=128 float32r matmul. d3/d5/d9 are
materialized on DVE, d7 on GPSIMD, and d11/d13/d15 are fed as two direct
column-shifted matmuls each (shifts are free in the rhs access pattern).
Matmuls are issued weight-major across each 2048-col window so the PE can
amortize/overlap weight loads and stay dense (HAM warm).

Sharding: rows split across 8 cores; halo rows come from host-side edge
padding so cores are fully independent.
"""

import numpy as np

import concourse.bass as bass
import concourse.mybir as mybir
import concourse.tile as tile
from concourse import bacc, bass_utils

H = W = 4096
NC = 8
RPC = H // NC                 # 512 output rows per core
PAD = 7
PW = W + 2 * PAD              # 4110 padded cols
PR = RPC + 2 * PAD            # 526 padded rows per core
M_TILE = 114                  # out rows per PE tile (114 + 14 = 128 contraction)
ROW_TILES = [(0, 114), (114, 114), (228, 114), (342, 114), (456, 56)]
CHUNK = 2048                  # column window for arrays + weight-major matmuls
K_SIZES = [3, 5, 7, 9, 11, 13, 15]
F32R = mybir.dt.float32r
F32 = mybir.dt.float32


def _weights_np() -> np.ndarray:
    """lhsT matrices [7, 128, M_TILE]: lhsT[j][i, m] = w_j[i - m]."""
    c = {k: 1.0 / (len(K_SIZES) * k * k) for k in K_SIZES}
    wts = np.zeros((7, 128, M_TILE), dtype=np.float64)
    for ji, j in enumerate(K_SIZES):
        w = np.array(
            [sum(c[k] for k in K_SIZES if k >= j and k >= 2 * abs(d - PAD) + 1)
             for d in range(2 * PAD + 1)])
        for m in range(M_TILE):
            wts[ji, m:m + 15, m] = w
    return wts.astype(np.float32)


def _kernel_body(nc, tc, xp_d, bm_d, w_d, out_d):
    add = mybir.AluOpType.add
    mult = mybir.AluOpType.mult

    with (
        tc.tile_pool(name="wpool", bufs=1) as wpool,
        tc.tile_pool(name="xpool", bufs=2) as xpool,
        tc.tile_pool(name="apool", bufs=2) as apool,
        tc.tile_pool(name="bmpool", bufs=2) as bmpool,
        tc.tile_pool(name="opool", bufs=2) as opool,
        tc.tile_pool(name="psum", bufs=2, space="PSUM") as psum_pool,
    ):
        wsb = wpool.tile([128, 7 * M_TILE], F32R)
        for j in range(7):
            nc.sync.dma_start(out=wsb[:, j * M_TILE:(j + 1) * M_TILE], in_=w_d[j])

        def wt(ji, Krows, Mt):
            return wsb[:Krows, ji * M_TILE:ji * M_TILE + Mt]

        for rt, Mt in ROW_TILES:
            Krows = min(128, PR - rt)     # 128, last tile 70
            x_sb = xpool.tile([128, PW], F32R, tag="x")
            nc.sync.dma_start(out=x_sb[:Krows], in_=xp_d[rt:rt + Krows])
            bm_sb = bmpool.tile([128, W], F32, tag="bm")
            nc.sync.dma_start(out=bm_sb[:Mt], in_=bm_d[rt:rt + Mt])
            X = x_sb[:Krows]
            osb = opool.tile([128, W], F32, tag="o")

            for co in range(0, W, CHUNK):
                # delta arrays for this column window
                d3 = apool.tile([128, CHUNK], F32R, tag="d3")
                d5 = apool.tile([128, CHUNK], F32R, tag="d5")
                d7 = apool.tile([128, CHUNK], F32R, tag="d7")
                d9 = apool.tile([128, CHUNK], F32R, tag="d9")
                nc.vector.tensor_tensor(
                    out=d3[:Krows], in0=X[:, co + 6:co + 6 + CHUNK],
                    in1=X[:, co + 8:co + 8 + CHUNK], op=add)
                nc.vector.tensor_tensor(
                    out=d5[:Krows], in0=X[:, co + 5:co + 5 + CHUNK],
                    in1=X[:, co + 9:co + 9 + CHUNK], op=add)
                nc.gpsimd.tensor_tensor(
                    out=d7[:Krows], in0=X[:, co + 4:co + 4 + CHUNK],
                    in1=X[:, co + 10:co + 10 + CHUNK], op=add)
                nc.vector.tensor_tensor(
                    out=d9[:Krows], in0=X[:, co + 3:co + 3 + CHUNK],
                    in1=X[:, co + 11:co + 11 + CHUNK], op=add)

                ps = psum_pool.tile([M_TILE, CHUNK], F32)
                nsl = CHUNK // 512

                def mms(ji, rhs_of, start=False, stop=False):
                    # one matmul per 512-slice of the window, same weight
                    for s in range(nsl):
                        nc.tensor.matmul(
                            ps[:Mt, s * 512:(s + 1) * 512],
                            wt(ji, Krows, Mt), rhs_of(s),
                            start=start, stop=stop)

                # weight-major over the window; d7 (gpsimd, slowest) last
                mms(0, lambda s: X[:, co + s * 512 + 7:co + s * 512 + 519],
                    start=True)
                mms(0, lambda s: d3[:Krows, s * 512:s * 512 + 512])
                mms(1, lambda s: d5[:Krows, s * 512:s * 512 + 512])
                mms(3, lambda s: d9[:Krows, s * 512:s * 512 + 512])
                for ji, p in ((4, 5), (5, 6), (6, 7)):
                    mms(ji, lambda s, p=p: X[:, co + s * 512 + 7 - p:
                                             co + s * 512 + 519 - p])
                    mms(ji, lambda s, p=p: X[:, co + s * 512 + 7 + p:
                                             co + s * 512 + 519 + p])
                mms(2, lambda s: d7[:Krows, s * 512:s * 512 + 512], stop=True)

                nc.vector.tensor_tensor(
                    out=osb[:Mt, co:co + CHUNK], in0=ps[:Mt],
                    in1=bm_sb[:Mt, co:co + CHUNK], op=mult)
            nc.sync.dma_start(out=out_d[rt:rt + Mt], in_=osb[:Mt])


def _build():
    nc = bacc.Bacc("TRN2", target_bir_lowering=False, debug=False)
    xp_d = nc.dram_tensor("xp", [PR, PW], F32R, kind="ExternalInput").ap()
    bm_d = nc.dram_tensor("bm", [RPC, W], F32, kind="ExternalInput").ap()
    w_d = nc.dram_tensor("wts", [7, 128, M_TILE], F32R, kind="ExternalInput").ap()
    out_d = nc.dram_tensor("out", [RPC, W], F32, kind="ExternalOutput").ap()
    with tile.TileContext(nc) as tc:
        _kernel_body(nc, tc, xp_d, bm_d, w_d, out_d)
    nc.compile()
    return nc


_CACHE: dict = {}


def _get_nc():
    if "nc" not in _CACHE:
        _CACHE["nc"] = _build()
    return _CACHE["nc"]


def _in_maps(x: np.ndarray, base_map: np.ndarray) -> list[dict]:
    xp = np.pad(x, PAD, mode="edge")
    wts = _weights_np()
    maps = []
    for c in range(NC):
        maps.append({
            "xp": np.ascontiguousarray(xp[c * RPC: c * RPC + PR]),
            "bm": np.ascontiguousarray(base_map[c * RPC:(c + 1) * RPC]),
            "wts": wts,
        })
    return maps


def run(x, base_map, **kwargs) -> tuple[np.ndarray, bass_utils.BassKernelResults]:
    x = np.ascontiguousarray(np.asarray(x), dtype=np.float32)
    base_map = np.ascontiguousarray(np.asarray(base_map), dtype=np.float32)
    nc = _get_nc()
    res = bass_utils.run_bass_kernel_spmd(
        nc, _in_maps(x, base_map), core_ids=list(range(NC)), **kwargs)
    out = np.concatenate([r["out"] for r in res.results], axis=0)
    return out[None, None].astype(np.float32, copy=False), res


def kernel(x, base_map) -> np.ndarray:
    return run(x, base_map)[0]


# revision 5
# speedup vs baseline: 1.1096x; 1.0168x over previous
"""Trainium2 Bass kernel for sum-of-7-box-blurs (k=3..15, edge padding) * base_map.

Math: out = base_map * sum_k 1/(7 k^2) * V_k(H_k(x)) with V_k/H_k k-wide box
sums (edge padding = clamped indexing, handled by host-side padding).

Horizontal delta decomposition (p = (j-1)/2, d_j = x<<p + x>>p column shifts):
  acc = M_3 x + sum_{j in 3..15 step 2} M_j d_j,   M_j = sum_{k>=j} c_k A_k
where M_j are 15-wide banded vertical matrices folded on the host. Per out
row-tile the vertical mix is a K=128 bf16 matmul accumulating in fp32 PSUM.
d3/d11/d15 are materialized on DVE (even col offsets -> bf16 2x mode), d7/d9
on GPSIMD, and x/d5/d13 are fed as direct column-shifted matmuls (shifts are
free in the rhs access pattern). Matmuls are issued weight-major across each
2048-col window so the PE amortizes weight loads and stays dense (HAM warm).
ScalarE evacuates PSUM; DVE does the base_map multiply in SBUF.

Sharding: rows split across 8 cores; halo rows come from host-side edge
padding so cores are fully independent.
"""

import numpy as np
import ml_dtypes

import concourse.bass as bass
import concourse.mybir as mybir
import concourse.tile as tile
from concourse import bacc, bass_utils

H = W = 4096
NC = 8
RPC = H // NC                 # 512 output rows per core
PAD = 7
PW = W + 2 * PAD              # 4110 padded cols
PR = RPC + 2 * PAD            # 526 padded rows per core
M_TILE = 114                  # valid out rows per PE tile (114 + 14 = 128)
ROW_TILES = [(0, 114), (114, 114), (228, 114), (342, 114), (456, 56)]
CHUNK = 2048                  # column window for arrays + weight-major matmuls
K_SIZES = [3, 5, 7, 9, 11, 13, 15]
BF16 = mybir.dt.bfloat16
F32 = mybir.dt.float32
NP_BF16 = ml_dtypes.bfloat16


def _weights_np() -> np.ndarray:
    """lhsT matrices [7, 128, 128]: lhsT[j][i, m] = w_j[i - m].

    m >= M_TILE columns produce partial sums for out-of-tile rows; they are
    never read. Full 128 weight columns enable fast weight load (FWL)."""
    c = {k: 1.0 / (len(K_SIZES) * k * k) for k in K_SIZES}
    wts = np.zeros((7, 128 + 2 * PAD, 128), dtype=np.float64)
    for ji, j in enumerate(K_SIZES):
        w = np.array(
            [sum(c[k] for k in K_SIZES if k >= j and k >= 2 * abs(d - PAD) + 1)
             for d in range(2 * PAD + 1)])
        for m in range(128):
            wts[ji, m:m + 15, m] = w
    return wts[:, :128, :].astype(NP_BF16)


def _kernel_body(nc, tc, xp_d, bm_d, w_d, out_d):
    add = mybir.AluOpType.add
    mult = mybir.AluOpType.mult

    with (
        tc.tile_pool(name="wpool", bufs=1) as wpool,
        tc.tile_pool(name="xpool", bufs=2) as xpool,
        tc.tile_pool(name="apool", bufs=2) as apool,
        tc.tile_pool(name="bmpool", bufs=2) as bmpool,
        tc.tile_pool(name="ppool", bufs=2) as ppool,
        tc.tile_pool(name="opool", bufs=3) as opool,
        tc.tile_pool(name="psum", bufs=2, space="PSUM") as psum_pool,
    ):
        wsb = wpool.tile([128, 7 * 128], BF16)
        for j in range(7):
            nc.sync.dma_start(out=wsb[:, j * 128:(j + 1) * 128], in_=w_d[j])

        def wt(ji, Krows):
            return wsb[:Krows, ji * 128:(ji + 1) * 128]

        # PE warmup: keep the HAM activity window busy during the initial
        # DMA fill so real matmuls start at full clock.
        warm = psum_pool.tile([128, CHUNK], F32, tag="ps")
        for i in range(24):
            s = i % 4
            nc.tensor.matmul(
                warm[:, s * 512:(s + 1) * 512], wsb[:, 0:128],
                wsb[:, 128:640], start=(i < 4), stop=(i >= 20))

        for rt, Mt in ROW_TILES:
            Krows = min(128, PR - rt)     # 128, last tile 70
            x_sb = xpool.tile([128, PW], BF16, tag="x")
            nc.sync.dma_start(out=x_sb[:Krows], in_=xp_d[rt:rt + Krows])
            bm_sb = bmpool.tile([128, W], F32, tag="bm")
            nc.sync.dma_start(out=bm_sb[:Mt], in_=bm_d[rt:rt + Mt])
            X = x_sb[:Krows]

            for co in range(0, W, CHUNK):
                # materialized delta arrays (even col offsets -> DVE 2x mode)
                d3 = apool.tile([128, CHUNK], BF16, tag="d3")
                d11 = apool.tile([128, CHUNK], BF16, tag="d11")
                d15 = apool.tile([128, CHUNK], BF16, tag="d15")
                d7 = apool.tile([128, CHUNK], BF16, tag="d7")
                d9 = apool.tile([128, CHUNK], BF16, tag="d9")
                nc.vector.tensor_tensor(
                    out=d3[:Krows], in0=X[:, co + 6:co + 6 + CHUNK],
                    in1=X[:, co + 8:co + 8 + CHUNK], op=add)
                nc.vector.tensor_tensor(
                    out=d11[:Krows], in0=X[:, co + 2:co + 2 + CHUNK],
                    in1=X[:, co + 12:co + 12 + CHUNK], op=add)
                nc.vector.tensor_tensor(
                    out=d15[:Krows], in0=X[:, co + 0:co + 0 + CHUNK],
                    in1=X[:, co + 14:co + 14 + CHUNK], op=add)
                nc.gpsimd.tensor_tensor(
                    out=d7[:Krows], in0=X[:, co + 4:co + 4 + CHUNK],
                    in1=X[:, co + 10:co + 10 + CHUNK], op=add)
                nc.gpsimd.tensor_tensor(
                    out=d9[:Krows], in0=X[:, co + 3:co + 3 + CHUNK],
                    in1=X[:, co + 11:co + 11 + CHUNK], op=add)

                ps = psum_pool.tile([128, CHUNK], F32, tag="ps")
                nsl = CHUNK // 512

                def mms(ji, rhs_of, start=False, stop=False):
                    for s in range(nsl):
                        nc.tensor.matmul(
                            ps[:, s * 512:(s + 1) * 512],
                            wt(ji, Krows), rhs_of(s), start=start, stop=stop)

                def xs(s, off):
                    base = co + s * 512 + off
                    return X[:, base:base + 512]

                # weight-major over the window; gpsimd-fed terms last
                mms(0, lambda s: xs(s, 7), start=True)          # x base
                mms(0, lambda s: d3[:Krows, s * 512:s * 512 + 512])
                mms(1, lambda s: xs(s, 5))                      # d5 pair
                mms(1, lambda s: xs(s, 9))
                mms(4, lambda s: d11[:Krows, s * 512:s * 512 + 512])
                mms(5, lambda s: xs(s, 1))                      # d13 pair
                mms(5, lambda s: xs(s, 13))
                mms(6, lambda s: d15[:Krows, s * 512:s * 512 + 512])
                mms(3, lambda s: d9[:Krows, s * 512:s * 512 + 512])
                mms(2, lambda s: d7[:Krows, s * 512:s * 512 + 512],
                    stop=True)

                # evacuate PSUM on ScalarE, multiply by base_map on DVE
                psc = ppool.tile([128, CHUNK], F32, tag="psc")
                nc.scalar.copy(out=psc[:Mt], in_=ps[:Mt])
                osb = opool.tile([128, CHUNK], F32, tag="o")
                nc.vector.tensor_tensor(
                    out=osb[:Mt], in0=psc[:Mt],
                    in1=bm_sb[:Mt, co:co + CHUNK], op=mult)
                nc.sync.dma_start(
                    out=out_d[rt:rt + Mt, co:co + CHUNK], in_=osb[:Mt])


def _build():
    nc = bacc.Bacc("TRN2", target_bir_lowering=False, debug=False)
    xp_d = nc.dram_tensor("xp", [PR, PW], BF16, kind="ExternalInput").ap()
    bm_d = nc.dram_tensor("bm", [RPC, W], F32, kind="ExternalInput").ap()
    w_d = nc.dram_tensor("wts", [7, 128, 128], BF16, kind="ExternalInput").ap()
    out_d = nc.dram_tensor("out", [RPC, W], F32, kind="ExternalOutput").ap()
    with tile.TileContext(nc) as tc:
        _kernel_body(nc, tc, xp_d, bm_d, w_d, out_d)
    nc.compile()
    return nc


_CACHE: dict = {}


def _get_nc():
    if "nc" not in _CACHE:
        _CACHE["nc"] = _build()
    return _CACHE["nc"]


def _in_maps(x: np.ndarray, base_map: np.ndarray) -> list[dict]:
    xp = np.pad(x, PAD, mode="edge").astype(NP_BF16)
    wts = _weights_np()
    maps = []
    for c in range(NC):
        maps.append({
            "xp": np.ascontiguousarray(xp[c * RPC: c * RPC + PR]),
            "bm": np.ascontiguousarray(base_map[c * RPC:(c + 1) * RPC]),
            "wts": wts,
        })
    return maps


def run(x, base_map, **kwargs) -> tuple[np.ndarray, bass_utils.BassKernelResults]:
    x = np.ascontiguousarray(np.asarray(x), dtype=np.float32)
    base_map = np.ascontiguousarray(np.asarray(base_map), dtype=np.float32)
    nc = _get_nc()
    res = bass_utils.run_bass_kernel_spmd(
        nc, _in_maps(x, base_map), core_ids=list(range(NC)), **kwargs)
    out = np.concatenate([r["out"] for r in res.results], axis=0)
    return out[None, None].astype(np.float32, copy=False), res


def kernel(x, base_map) -> np.ndarray:
    return run(x, base_map)[0]


# revision 6
# speedup vs baseline: 1.3778x; 1.2417x over previous
"""Trainium2 Bass kernel for sum-of-7-box-blurs (k=3..15, edge padding) * base_map.

Math: out = base_map * sum_k 1/(7 k^2) * V_k(H_k(x)) with V_k/H_k k-wide box
sums (edge padding = clamped indexing, handled by host-side padding).

Horizontal delta decomposition (p = (j-1)/2, d_j = x<<p + x>>p column shifts):
  acc = M_3 x + sum_{j in 3..15 step 2} M_j d_j,   M_j = sum_{k>=j} c_k A_k
where M_j are 15-wide banded vertical matrices folded on the host. Per out
row-tile the vertical mix is a K=128 bf16 matmul accumulating in fp32 PSUM.
d3/d11/d15 are materialized on DVE (even col offsets -> bf16 2x mode), d7/d9
on GPSIMD, and x/d5/d13 are fed as direct column-shifted matmuls (shifts are
free in the rhs access pattern). Matmuls are issued weight-major across each
2048-col window so the PE amortizes weight loads and stays dense (HAM warm).
ScalarE evacuates PSUM; DVE does the base_map multiply in SBUF.

Sharding: rows split across 8 cores; halo rows come from host-side edge
padding so cores are fully independent.
"""

import numpy as np
import ml_dtypes

import concourse.bass as bass
import concourse.mybir as mybir
import concourse.tile as tile
from concourse import bacc, bass_utils

H = W = 4096
NC = 8
RPC = H // NC                 # 512 output rows per core
PAD = 7
PW = W + 2 * PAD              # 4110 padded cols
PR = RPC + 2 * PAD            # 526 padded rows per core
M_TILE = 114                  # valid out rows per PE tile (114 + 14 = 128)
ROW_TILES = [(0, 114), (114, 114), (228, 114), (342, 114), (456, 56)]
CHUNK = 2048                  # column window for arrays + weight-major matmuls
K_SIZES = [3, 5, 7, 9, 11, 13, 15]
BF16 = mybir.dt.bfloat16
F32 = mybir.dt.float32
NP_BF16 = ml_dtypes.bfloat16


def _weights_np() -> np.ndarray:
    """lhsT matrices [7, 128, 128]: lhsT[j][i, m] = w_j[i - m].

    m >= M_TILE columns produce partial sums for out-of-tile rows; they are
    never read. Full 128 weight columns enable fast weight load (FWL)."""
    c = {k: 1.0 / (len(K_SIZES) * k * k) for k in K_SIZES}
    wts = np.zeros((7, 128 + 2 * PAD, 128), dtype=np.float64)
    for ji, j in enumerate(K_SIZES):
        w = np.array(
            [sum(c[k] for k in K_SIZES if k >= j and k >= 2 * abs(d - PAD) + 1)
             for d in range(2 * PAD + 1)])
        for m in range(128):
            wts[ji, m:m + 15, m] = w
    return wts[:, :128, :].astype(NP_BF16)


def _kernel_body(nc, tc, xp_d, bm_d, w_d, out_d):
    add = mybir.AluOpType.add
    mult = mybir.AluOpType.mult

    with (
        tc.tile_pool(name="wpool", bufs=1) as wpool,
        tc.tile_pool(name="xpool", bufs=3) as xpool,
        tc.tile_pool(name="apool", bufs=2) as apool,
        tc.tile_pool(name="bmpool", bufs=2) as bmpool,
        tc.tile_pool(name="ppool", bufs=2) as ppool,
        tc.tile_pool(name="opool", bufs=3) as opool,
        tc.tile_pool(name="psum", bufs=2, space="PSUM") as psum_pool,
    ):
        wsb = wpool.tile([128, 7 * 128], BF16)
        for j in range(7):
            nc.sync.dma_start(out=wsb[:, j * 128:(j + 1) * 128], in_=w_d[j])

        def wt(ji, Krows):
            return wsb[:Krows, ji * 128:(ji + 1) * 128]

        # PE warmup: keep the HAM activity window busy during the initial
        # DMA fill so real matmuls start at full clock.
        warm = psum_pool.tile([128, CHUNK], F32, tag="ps")
        for i in range(32):
            s = i % 4
            nc.tensor.matmul(
                warm[:, s * 512:(s + 1) * 512], wsb[:, 0:128],
                wsb[:, 128:640], start=(i < 4), stop=(i >= 28))

        for rt, Mt in ROW_TILES:
            Krows = min(128, PR - rt)     # 128, last tile 70
            x_sb = xpool.tile([128, PW], BF16, tag="x")
            nc.sync.dma_start(out=x_sb[:Krows], in_=xp_d[rt:rt + Krows])
            bm_sb = bmpool.tile([128, W], F32, tag="bm")
            nc.sync.dma_start(out=bm_sb[:Mt], in_=bm_d[rt:rt + Mt])
            X = x_sb[:Krows]

            for co in range(0, W, CHUNK):
                # materialized delta arrays (even col offsets -> DVE 2x mode)
                d3 = apool.tile([128, CHUNK], BF16, tag="d3")
                d11 = apool.tile([128, CHUNK], BF16, tag="d11")
                d15 = apool.tile([128, CHUNK], BF16, tag="d15")
                d7 = apool.tile([128, CHUNK], BF16, tag="d7")
                d9 = apool.tile([128, CHUNK], BF16, tag="d9")
                nc.vector.tensor_tensor(
                    out=d3[:Krows], in0=X[:, co + 6:co + 6 + CHUNK],
                    in1=X[:, co + 8:co + 8 + CHUNK], op=add)
                nc.vector.tensor_tensor(
                    out=d11[:Krows], in0=X[:, co + 2:co + 2 + CHUNK],
                    in1=X[:, co + 12:co + 12 + CHUNK], op=add)
                nc.vector.tensor_tensor(
                    out=d15[:Krows], in0=X[:, co + 0:co + 0 + CHUNK],
                    in1=X[:, co + 14:co + 14 + CHUNK], op=add)
                nc.gpsimd.tensor_tensor(
                    out=d7[:Krows], in0=X[:, co + 4:co + 4 + CHUNK],
                    in1=X[:, co + 10:co + 10 + CHUNK], op=add)
                nc.vector.tensor_tensor(
                    out=d9[:Krows], in0=X[:, co + 3:co + 3 + CHUNK],
                    in1=X[:, co + 11:co + 11 + CHUNK], op=add)

                ps = psum_pool.tile([128, CHUNK], F32, tag="ps")
                nsl = CHUNK // 512

                def mms(ji, rhs_of, start=False, stop=False):
                    for s in range(nsl):
                        nc.tensor.matmul(
                            ps[:, s * 512:(s + 1) * 512],
                            wt(ji, Krows), rhs_of(s), start=start, stop=stop)

                def xs(s, off):
                    base = co + s * 512 + off
                    return X[:, base:base + 512]

                # weight-major over the window; gpsimd-fed terms last
                mms(0, lambda s: xs(s, 7), start=True)          # x base
                mms(0, lambda s: d3[:Krows, s * 512:s * 512 + 512])
                mms(1, lambda s: xs(s, 5))                      # d5 pair
                mms(1, lambda s: xs(s, 9))
                mms(4, lambda s: d11[:Krows, s * 512:s * 512 + 512])
                mms(5, lambda s: xs(s, 1))                      # d13 pair
                mms(5, lambda s: xs(s, 13))
                mms(6, lambda s: d15[:Krows, s * 512:s * 512 + 512])
                mms(3, lambda s: d9[:Krows, s * 512:s * 512 + 512])
                mms(2, lambda s: d7[:Krows, s * 512:s * 512 + 512],
                    stop=True)

                # evacuate PSUM on ScalarE, multiply by base_map on DVE
                psc = ppool.tile([128, CHUNK], F32, tag="psc")
                nc.scalar.copy(out=psc[:Mt], in_=ps[:Mt])
                osb = opool.tile([128, CHUNK], F32, tag="o")
                nc.vector.tensor_tensor(
                    out=osb[:Mt], in0=psc[:Mt],
                    in1=bm_sb[:Mt, co:co + CHUNK], op=mult)
                nc.sync.dma_start(
                    out=out_d[rt:rt + Mt, co:co + CHUNK], in_=osb[:Mt])


def _build():
    nc = bacc.Bacc("TRN2", target_bir_lowering=False, debug=False)
    xp_d = nc.dram_tensor("xp", [PR, PW], BF16, kind="ExternalInput").ap()
    bm_d = nc.dram_tensor("bm", [RPC, W], F32, kind="ExternalInput").ap()
    w_d = nc.dram_tensor("wts", [7, 128, 128], BF16, kind="ExternalInput").ap()
    out_d = nc.dram_tensor("out", [RPC, W], F32, kind="ExternalOutput").ap()
    with tile.TileContext(nc) as tc:
        _kernel_body(nc, tc, xp_d, bm_d, w_d, out_d)
    nc.compile()
    return nc


_CACHE: dict = {}


def _get_nc():
    if "nc" not in _CACHE:
        _CACHE["nc"] = _build()
    return _CACHE["nc"]


def _in_maps(x: np.ndarray, base_map: np.ndarray) -> list[dict]:
    xp = np.pad(x, PAD, mode="edge").astype(NP_BF16)
    wts = _weights_np()
    maps = []
    for c in range(NC):
        maps.append({
            "xp": np.ascontiguousarray(xp[c * RPC: c * RPC + PR]),
            "bm": np.ascontiguousarray(base_map[c * RPC:(c + 1) * RPC]),
            "wts": wts,
        })
    return maps


def run(x, base_map, **kwargs) -> tuple[np.ndarray, bass_utils.BassKernelResults]:
    x = np.ascontiguousarray(np.asarray(x), dtype=np.float32)
    base_map = np.ascontiguousarray(np.asarray(base_map), dtype=np.float32)
    nc = _get_nc()
    res = bass_utils.run_bass_kernel_spmd(
        nc, _in_maps(x, base_map), core_ids=list(range(NC)), **kwargs)
    out = np.concatenate([r["out"] for r in res.results], axis=0)
    return out[None, None].astype(np.float32, copy=False), res


def kernel(x, base_map) -> np.ndarray:
    return run(x, base_map)[0]
